# revision 10
# baseline (speedup 1.0000x reference)
"""Trainium2 Bass kernel for a 3-layer GCN corrector (AccessibilityGNNCorrector).

Strategy: node-parallel across 8 NeuronCores (12500 dst nodes per core).
Per GCN layer each core computes its shard of hw = h @ W, AllGathers the
full [N, F] gather table, then aggregates its incoming edges with
dma_gather (rows by src) + one-hot selection-matrix matmuls into PSUM.
BatchNorm statistics are AllReduced ([128,2] per layer). Edge metadata
(int16 gather indices, dst-local one-hot columns, edge norms) is
precomputed on the host as part of graph partitioning.
"""

import numpy as np

import concourse.bacc as bacc
import concourse.mybir as mybir
import concourse.tile as tile
from concourse import library_config
from concourse.bass_utils import run_bass_kernel_spmd

F32 = mybir.dt.float32
I16 = mybir.dt.int16
AF = mybir.ActivationFunctionType
ALU = mybir.AluOpType

TILE = 128          # dst nodes per aggregation tile (PSUM partition dim)
T_MERGE = 2         # dst tiles merged per dma_gather call
APPLY_W = 512       # free-dim width for BN-apply / table-build chunks
N_GROUPS = 4        # src buckets (src & 3) so int16 gather indices fit


# --------------------------------------------------------------------------
# host-side graph partitioning / metadata layout
# --------------------------------------------------------------------------

def _layout_sections(sec_pad, n_tiles):
    """Iteration order shared by host packing and device program.

    Returns list of t4-groups:
      {tiles: [t..], calls: [{g, K, token_base, secs: [(t, n_chunks,
       chunk_off_in_call)]}], chunk_base, n_chunks}
    token positions are global over the concatenation of all calls.
    """
    t4s = []
    tok = 0
    for t0 in range(0, n_tiles, T_MERGE):
        tiles = list(range(t0, min(t0 + T_MERGE, n_tiles)))
        grp = {"tiles": tiles, "calls": [], "chunk_base": tok // TILE}
        for g in range(N_GROUPS):
            K = int(sum(sec_pad[t][g] for t in tiles))
            if K == 0:
                continue
            call = {"g": g, "K": K, "token_base": tok, "secs": []}
            off = 0
            for t in tiles:
                n_ch = sec_pad[t][g] // TILE
                if n_ch:
                    call["secs"].append((t, n_ch, off))
                off += n_ch
            grp["calls"].append(call)
            tok += K
        grp["n_chunks"] = tok // TILE - grp["chunk_base"]
        t4s.append(grp)
    return t4s, tok


def _prepare(x, edge_index, n_cores):
    n = x.shape[0]
    e = edge_index.shape[1]
    assert n % n_cores == 0
    sh = n // n_cores
    n_tiles = (sh + TILE - 1) // TILE
    sh_pad = n_tiles * TILE

    src = np.ascontiguousarray(edge_index[0]).astype(np.int64)
    dst = np.ascontiguousarray(edge_index[1]).astype(np.int64)
    deg = (np.bincount(dst, minlength=n) + 1.0).astype(np.float32)
    dinv = (1.0 / np.sqrt(deg)).astype(np.float32)

    # append self loops (norm = dinv^2), then per-edge norm
    loops = np.arange(n, dtype=np.int64)
    src_a = np.concatenate([src, loops])
    dst_a = np.concatenate([dst, loops])
    nrm_a = (dinv[src_a] * dinv[dst_a]).astype(np.float32)

    core_of = dst_a // sh
    per_core = []
    cnts = np.zeros((n_cores, n_tiles, N_GROUPS), np.int64)
    for k in range(n_cores):
        m = core_of == k
        s, d, w = src_a[m], dst_a[m] - k * sh, nrm_a[m]
        t = d >> 7
        g = (s & (N_GROUPS - 1)).astype(np.int64)
        order = np.lexsort((s, g, t))
        s, d, w, t, g = s[order], d[order], w[order], t[order], g[order]
        np.add.at(cnts[k], (t, g), 1)
        per_core.append((s, d, w, t, g))

    cmax = cnts.max(axis=0)
    sec_pad = ((cmax + TILE - 1) // TILE * TILE).astype(np.int64)
    t4s, total_tok = _layout_sections(sec_pad, n_tiles)
    total_chunks = total_tok // TILE

    # global token base per (t, g) section
    sec_base = np.zeros((n_tiles, N_GROUPS), np.int64)
    for grp in t4s:
        for call in grp["calls"]:
            for (t, n_ch, off) in call["secs"]:
                sec_base[t][call["g"]] = call["token_base"] + off * TILE

    idx_cols = total_tok // 16
    core_inputs = []
    for k in range(n_cores):
        s, d, w, t, g = per_core[k]
        cnt = cnts[k]
        # destination slot per edge: sec_base[t,g] + rank within section
        sec_id = t * N_GROUPS + g
        flat_cnt = cnt.reshape(-1)
        sec_start_edge = np.concatenate([[0], np.cumsum(flat_cnt)])[:-1]
        rank = np.arange(len(s)) - sec_start_edge[sec_id]
        dest = sec_base.reshape(-1)[sec_id] + rank

        tok_idx = np.zeros(total_tok, np.int16)
        tok_dloc = np.zeros(total_tok, np.float32)
        tok_nrm = np.zeros(total_tok, np.float32)
        tok_idx[dest] = (s >> 2).astype(np.int16)
        tok_dloc[dest] = (d & (TILE - 1)).astype(np.float32)
        tok_nrm[dest] = w

        # idx dram: per call wrap to [16, K/16] then replicate to 128 rows
        idx_arr = np.zeros((128, idx_cols), np.int16)
        for grp in t4s:
            for call in grp["calls"]:
                a, K = call["token_base"], call["K"]
                wrp = tok_idx[a:a + K].reshape(-1, 16).T  # [16, K/16]
                idx_arr[:, a // 16:(a + K) // 16] = np.tile(wrp, (8, 1))
        # meta dram: token (chunk c, part p) -> [p, c]
        dloc_arr = tok_dloc.reshape(total_chunks, TILE).T.copy()
        nrm_arr = tok_nrm.reshape(total_chunks, TILE).T.copy()

        xT = np.zeros((x.shape[1], sh_pad), np.float32)
        xT[:, :sh] = x[k * sh:(k + 1) * sh].T
        core_inputs.append({
            "xT": xT, "idx": idx_arr, "dloc": dloc_arr, "nrm": nrm_arr,
        })

    plan = {
        "n": n, "e": e, "n_cores": n_cores, "sh": sh, "sh_pad": sh_pad,
        "n_tiles": n_tiles, "t4s": t4s, "total_tok": total_tok,
        "total_chunks": total_chunks, "idx_cols": idx_cols,
        "in_dim": x.shape[1],
    }
    return core_inputs, plan


def _weight_inputs(inputs, plan):
    """Weight/param arrays (replicated on every core)."""
    hid = inputs["W1"].shape[0]
    f3 = inputs["W3"].shape[1]
    smalls = np.zeros((128, 8), np.float32)
    smalls[:hid, 0] = inputs["bp"]
    smalls[:hid, 1] = inputs["gamma1"]
    smalls[:hid, 2] = inputs["beta1"]
    smalls[:hid, 3] = inputs["gamma2"]
    smalls[:hid, 4] = inputs["beta2"]
    smalls[:f3, 5] = inputs["b3"]
    smalls[0, 6] = inputs["bh"][0]
    smalls[:, 7] = 1e-5
    wp = np.asarray(inputs["Wp"], np.float32)
    return {
        "Wpa": np.ascontiguousarray(wp[:128]),
        "Wpb": np.ascontiguousarray(wp[128:256]),
        "W1": np.asarray(inputs["W1"], np.float32),
        "W2": np.asarray(inputs["W2"], np.float32),
        "W3": np.asarray(inputs["W3"], np.float32),
        "Wh": np.asarray(inputs["Wh"], np.float32),
        "smalls": smalls,
    }


# --------------------------------------------------------------------------
# device program
# --------------------------------------------------------------------------

def _apply_chunks(sh_pad):
    out = []
    c = 0
    while c < sh_pad:
        w = min(APPLY_W, sh_pad - c)
        out.append((c, w))
        c += w
    return out


def build_program(plan, scale_const, hid=128, f3=64, eps=1e-5):
    n, sh, sh_pad = plan["n"], plan["sh"], plan["sh_pad"]
    n_cores, in_dim = plan["n_cores"], plan["in_dim"]
    t4s = plan["t4s"]
    groups = [list(range(n_cores))]
    inv_n = 1.0 / n

    nc = bacc.Bacc("TRN2", target_bir_lowering=False, debug=False,
                   num_devices=n_cores, num_swdge_queues=4)

    # I/O
    xT_d = nc.dram_tensor("xT", [in_dim, sh_pad], F32, kind="ExternalInput")
    idx_d = nc.dram_tensor("idx", [128, plan["idx_cols"]], I16,
                           kind="ExternalInput")
    dloc_d = nc.dram_tensor("dloc", [128, plan["total_chunks"]], F32,
                            kind="ExternalInput")
    nrm_d = nc.dram_tensor("nrm", [128, plan["total_chunks"]], F32,
                           kind="ExternalInput")
    Wpa_d = nc.dram_tensor("Wpa", [128, hid], F32, kind="ExternalInput")
    Wpb_d = nc.dram_tensor("Wpb", [128, hid], F32, kind="ExternalInput")
    W1_d = nc.dram_tensor("W1", [hid, hid], F32, kind="ExternalInput")
    W2_d = nc.dram_tensor("W2", [hid, hid], F32, kind="ExternalInput")
    W3_d = nc.dram_tensor("W3", [hid, f3], F32, kind="ExternalInput")
    Wh_d = nc.dram_tensor("Wh", [f3, 1], F32, kind="ExternalInput")
    smalls_d = nc.dram_tensor("smalls", [128, 8], F32, kind="ExternalInput")
    out_d = nc.dram_tensor("out", [1, sh], F32, kind="ExternalOutput")

    # internal scratch
    hw_shard = [nc.dram_tensor(f"hw{i}s", [sh, f], F32)
                for i, f in ((1, hid), (2, hid), (3, f3))]
    hw_full = [nc.dram_tensor(f"hw{i}f", [n, f], F32, addr_space="Shared")
               for i, f in ((1, hid), (2, hid), (3, f3))]
    agg_d = [nc.dram_tensor(f"agg{i}", [128, sh_pad], F32) for i in (1, 2)]
    h1T_d = nc.dram_tensor("h1T", [128, sh_pad], F32)
    stl_d = [nc.dram_tensor(f"stl{i}", [128, 2], F32) for i in (1, 2)]
    stg_d = [nc.dram_tensor(f"stg{i}", [128, 2], F32, addr_space="Shared")
             for i in (1, 2)]

    with tile.TileContext(nc) as tc:
        with (
            tc.tile_pool(name="const", bufs=1) as constp,
            tc.tile_pool(name="gbuf", bufs=3 * N_GROUPS) as gpool,
            tc.tile_pool(name="big", bufs=2) as bigpool,
            tc.tile_pool(name="spool", bufs=12) as spool,
            tc.tile_pool(name="meta", bufs=3) as metapool,
            tc.tile_pool(name="small", bufs=6) as smallpool,
            tc.tile_pool(name="psA", bufs=2, space="PSUM") as psA,
            tc.tile_pool(name="psB", bufs=3, space="PSUM") as psB,
            tc.tile_pool(name="psT", bufs=2, space="PSUM") as psT,
        ):
            nc.gpsimd.load_library(library_config.mlp)

            # ---- constants ----
            iota = constp.tile([128, 128], F32, tag="iota", name="iota")
            nc.gpsimd.iota(iota[:], pattern=[[1, 128]], base=0,
                           channel_multiplier=0,
                           allow_small_or_imprecise_dtypes=True)
            iota_c = constp.tile([128, 1], F32, tag="iota_c", name="iota_c")
            nc.gpsimd.iota(iota_c[:], pattern=[[0, 1]], base=0,
                           channel_multiplier=1,
                           allow_small_or_imprecise_dtypes=True)
            eye = constp.tile([128, 128], F32, tag="eye", name="eye")
            nc.vector.tensor_scalar(eye[:], iota[:], iota_c[:], None,
                                    ALU.is_equal)

            def load_const(name, dram, shape):
                t = constp.tile(shape, F32, tag=name)
                nc.sync.dma_start(t[:], dram[:])
                return t

            Wpa = load_const("Wpa", Wpa_d, [128, hid])
            Wpb = load_const("Wpb", Wpb_d, [128, hid])
            W1 = load_const("W1", W1_d, [hid, hid])
            W2 = load_const("W2", W2_d, [hid, hid])
            W3 = load_const("W3", W3_d, [hid, f3])
            Wh = load_const("Wh", Wh_d, [f3, 1])
            smalls = load_const("smalls", smalls_d, [128, 8])

            stats_sum = [constp.tile([128, plan["n_tiles"]], F32,
                                     tag=f"ssum{i}", name=f"ssum{i}") for i in (0, 1)]
            stats_sq = [constp.tile([128, plan["n_tiles"]], F32,
                                    tag=f"ssq{i}", name=f"ssq{i}") for i in (0, 1)]

            # ---- helper: write row-major table rows from T-land chunk ----
            def emit_table_rows(hwsb, fo, c0, w, shard):
                for j in range(w // 128):
                    r0 = c0 + j * 128
                    rows = min(128, sh - r0)
                    if rows <= 0:
                        continue
                    pst = psT.tile([128, fo], F32, tag="pst", name="pst")
                    nc.tensor.transpose(pst[:], hwsb[:fo, j * 128:(j + 1) * 128],
                                        eye[:fo, :fo])
                    rsb = smallpool.tile([128, fo], F32, tag="row", name="row")
                    nc.vector.tensor_copy(rsb[:], pst[:])
                    nc.sync.dma_start(shard[r0:r0 + rows, :], rsb[:rows, :])

            # ---- stage 0: projection + table 1 ----
            bp_ap = smalls[:hid, 0:1]
            for (c0, w) in _apply_chunks(sh_pad):
                xa = bigpool.tile([128, w], F32, tag="xa", name="xa")
                nc.sync.dma_start(xa[:], xT_d[0:128, c0:c0 + w])
                xb = bigpool.tile([128, w], F32, tag="xb", name="xb")
                nc.sync.dma_start(xb[:], xT_d[128:256, c0:c0 + w])
                ps = psB.tile([128, w], F32, tag="psb", name="psb")
                nc.tensor.matmul(ps[:], Wpa[:], xa[:], start=True, stop=False)
                nc.tensor.matmul(ps[:], Wpb[:], xb[:], start=False, stop=True)
                h0 = bigpool.tile([128, w], F32, tag="hsb", name="hsb")
                nc.scalar.activation(h0[:], ps[:], AF.Relu, bias=bp_ap)
                ps2 = psB.tile([128, w], F32, tag="psb", name="psb")
                nc.tensor.matmul(ps2[:], W1[:], h0[:], start=True, stop=True)
                hwsb = bigpool.tile([128, w], F32, tag="hw", name="hw")
                nc.scalar.copy(hwsb[:], ps2[:])
                emit_table_rows(hwsb, hid, c0, w, hw_shard[0])

            nc.gpsimd.collective_compute(
                "AllGather", ALU.bypass, replica_groups=groups,
                ins=[hw_shard[0][:]], outs=[hw_full[0][:]])

            # ---- aggregation pass (layers 1..3) ----
            def agg_pass(li):
                F = hid if li < 3 else f3
                table = hw_full[li - 1]
                tview = table[:].rearrange("(q four) f -> four q f", four=4)
                for grp in t4s:
                    cb = grp["chunk_base"]
                    ic0 = grp["calls"][0]["token_base"] // 16 if grp["calls"] else 0
                    icw = sum(c["K"] for c in grp["calls"]) // 16
                    islab = metapool.tile([128, max(icw, 1)], I16, tag="islab", name="islab")
                    if icw:
                        nc.sync.dma_start(islab[:, :icw],
                                          idx_d[:, ic0:ic0 + icw])
                    mw = grp["n_chunks"]
                    dslab = metapool.tile([128, max(mw, 1)], F32, tag="dslab", name="dslab")
                    nslab = metapool.tile([128, max(mw, 1)], F32, tag="nslab", name="nslab")
                    if mw:
                        nc.sync.dma_start(dslab[:, :mw], dloc_d[:, cb:cb + mw])
                        nc.sync.dma_start(nslab[:, :mw], nrm_d[:, cb:cb + mw])
                    gts = {}
                    for call in grp["calls"]:
                        K, g = call["K"], call["g"]
                        gt = gpool.tile([128, K // TILE, F], F32, tag="g", name="g")
                        lo = (call["token_base"] - grp["calls"][0]["token_base"]) // 16
                        assert K <= 8192, K
                        nc.gpsimd.dma_gather(
                            gt[:], tview[g], islab[:, lo:lo + K // 16],
                            K, K, F, elem_step=4 * F, single_packet=False,
                            queue_num=g)
                        gts[call["g"]] = (gt, call)
                    for t in grp["tiles"]:
                        mms = []
                        for g in sorted(gts):
                            gt, call = gts[g]
                            for (tt, n_ch, off) in call["secs"]:
                                if tt != t:
                                    continue
                                gcol = (call["token_base"] // TILE - cb) + off
                                for c in range(n_ch):
                                    mms.append((gt, off + c, gcol + c))
                        ps = psA.tile([F, 128], F32, tag="psa", name="psa")
                        for i, (gt, gc, mc) in enumerate(mms):
                            S = spool.tile([128, 128], F32, tag="S", name="S")
                            nc.vector.tensor_scalar(
                                S[:], iota[:], dslab[:, mc:mc + 1],
                                nslab[:, mc:mc + 1], ALU.is_equal, ALU.mult)
                            nc.tensor.matmul(ps[:], gt[:, gc, :], S[:],
                                             start=(i == 0),
                                             stop=(i == len(mms) - 1))
                        yield t, ps

            # ---- layers 1 and 2 ----
            for li in (1, 2):
                ssum, ssq = stats_sum[li - 1], stats_sq[li - 1]
                for t, ps in agg_pass(li):
                    asb = smallpool.tile([128, 128], F32, tag="asb", name="asb")
                    nc.scalar.copy(asb[:], ps[:])
                    nc.vector.tensor_reduce(ssum[:, t:t + 1], asb[:],
                                            mybir.AxisListType.X, ALU.add)
                    sq = smallpool.tile([128, 128], F32, tag="sq", name="sq")
                    nc.scalar.square(sq[:], asb[:])
                    nc.vector.tensor_reduce(ssq[:, t:t + 1], sq[:],
                                            mybir.AxisListType.X, ALU.add)
                    nc.sync.dma_start(agg_d[li - 1][:, t * 128:(t + 1) * 128],
                                      asb[:])
                # stats -> AllReduce
                st = smallpool.tile([128, 2], F32, tag="stpack", name="stpack")
                nc.vector.tensor_reduce(st[:, 0:1], ssum[:],
                                        mybir.AxisListType.X, ALU.add)
                nc.vector.tensor_reduce(st[:, 1:2], ssq[:],
                                        mybir.AxisListType.X, ALU.add)
                nc.sync.dma_start(stl_d[li - 1][:], st[:])
                nc.gpsimd.collective_compute(
                    "AllReduce", ALU.add, replica_groups=groups,
                    ins=[stl_d[li - 1][:]], outs=[stg_d[li - 1][:]])
                ssb = smallpool.tile([128, 2], F32, tag="ssb", name="ssb")
                nc.sync.dma_start(ssb[:], stg_d[li - 1][:])

                # BN scale/bias
                scr = smallpool.tile([128, 6], F32, tag="bnscr", name="bnscr")
                nc.vector.tensor_scalar(scr[:, 0:1], ssb[:, 0:1], inv_n, None,
                                        ALU.mult)                      # mean
                nc.vector.tensor_scalar(scr[:, 1:2], ssb[:, 1:2], inv_n, None,
                                        ALU.mult)                      # E[x^2]
                nc.vector.tensor_tensor(scr[:, 2:3], scr[:, 0:1], scr[:, 0:1],
                                        ALU.mult)                      # mean^2
                nc.vector.tensor_tensor(scr[:, 3:4], scr[:, 1:2], scr[:, 2:3],
                                        ALU.subtract)                  # var
                nc.scalar.activation(scr[:, 4:5], scr[:, 3:4], AF.Sqrt,
                                     bias=smalls[:, 7:8])                         # std
                inv_t = smallpool.tile([128, 1], F32, tag="invstd", name="invstd")
                nc.vector.reciprocal(inv_t[:], scr[:, 4:5])
                gamma_ap = smalls[:, 2 * li - 1:2 * li]
                beta_ap = smalls[:, 2 * li:2 * li + 1]
                scale_bn = constp.tile([128, 1], F32, tag=f"scale{li}", name=f"scale{li}")
                bias_bn = constp.tile([128, 1], F32, tag=f"bias{li}", name=f"bias{li}")
                nc.vector.tensor_tensor(scale_bn[:], inv_t[:], gamma_ap,
                                        ALU.mult)
                mb = smallpool.tile([128, 1], F32, tag="mb", name="mb")
                nc.vector.tensor_tensor(mb[:], scr[:, 0:1], scale_bn[:],
                                        ALU.mult)
                nc.vector.tensor_tensor(bias_bn[:], beta_ap, mb[:],
                                        ALU.subtract)

                # BN apply + next table build
                Wn = W2 if li == 1 else W3
                fo = hid if li == 1 else f3
                for (c0, w) in _apply_chunks(sh_pad):
                    asb2 = bigpool.tile([128, w], F32, tag="xa", name="xa")
                    nc.sync.dma_start(asb2[:], agg_d[li - 1][:, c0:c0 + w])
                    hsb = bigpool.tile([128, w], F32, tag="hsb", name="hsb")
                    nc.scalar.activation(hsb[:], asb2[:], AF.Relu,
                                         bias=bias_bn[:], scale=scale_bn[:])
                    if li == 2:
                        rsb = bigpool.tile([128, w], F32, tag="xb", name="xb")
                        nc.sync.dma_start(rsb[:], h1T_d[:, c0:c0 + w])
                        nc.vector.tensor_add(hsb[:], hsb[:], rsb[:])
                    else:
                        nc.sync.dma_start(h1T_d[:, c0:c0 + w], hsb[:])
                    psm = psB.tile([fo, w], F32, tag="psb", name="psb")
                    nc.tensor.matmul(psm[:], Wn[:, :fo], hsb[:],
                                     start=True, stop=True)
                    hwsb = bigpool.tile([128, w], F32, tag="hw", name="hw")
                    nc.scalar.copy(hwsb[:fo, :], psm[:])
                    emit_table_rows(hwsb, fo, c0, w, hw_shard[li])
                nc.gpsimd.collective_compute(
                    "AllGather", ALU.bypass, replica_groups=groups,
                    ins=[hw_shard[li][:]], outs=[hw_full[li][:]])

            # ---- layer 3 + head ----
            b3_ap = smalls[:f3, 5:6]
            bh_ap = smalls[0:1, 6:7]
            for t, ps in agg_pass(3):
                h3 = smallpool.tile([f3, 128], F32, tag="h3", name="h3")
                nc.scalar.activation(h3[:], ps[:], AF.Relu, bias=b3_ap)
                psh = psT.tile([1, 128], F32, tag="pst", name="psh")
                nc.tensor.matmul(psh[:], Wh[:], h3[:], start=True, stop=True)
                th = smallpool.tile([1, 128], F32, tag="th", name="th")
                nc.scalar.activation(th[:], psh[:], AF.Tanh, bias=bh_ap)
                ow = smallpool.tile([1, 128], F32, tag="ow", name="ow")
                nc.scalar.mul(ow[:], th[:], float(scale_const))
                w_out = min(128, sh - t * 128)
                nc.sync.dma_start(out_d[0:1, t * 128:t * 128 + w_out],
                                  ow[0:1, :w_out])

    nc.compile()
    return nc


# --------------------------------------------------------------------------
# entry point
# --------------------------------------------------------------------------

_CACHE = {}


def _build_all(inputs, n_cores=8):
    x = np.asarray(inputs["x"], np.float32)
    ei = np.asarray(inputs["edge_index"])
    key = (x.shape, ei.shape, float(np.asarray(inputs["scale"])))
    core_inputs, plan = _prepare(x, ei, n_cores)
    wmap = _weight_inputs(inputs, plan)
    in_maps = [{**ci, **wmap} for ci in core_inputs]
    if key in _CACHE:
        nc = _CACHE[key]
    else:
        nc = build_program(plan, float(np.asarray(inputs["scale"])),
                           hid=inputs["W1"].shape[0],
                           f3=inputs["W3"].shape[1])
        _CACHE[key] = nc
    return nc, in_maps, plan


def kernel(**inputs) -> np.ndarray:
    n_cores = 8
    nc, in_maps, plan = _build_all(inputs, n_cores)
    res = run_bass_kernel_spmd(nc, in_maps, list(range(n_cores)))
    outs = [res.results[k]["out"].reshape(-1) for k in range(n_cores)]
    return np.concatenate(outs).reshape(-1, 1).astype(np.float32)
